# revision 29
# baseline (speedup 1.0000x reference)
"""GQA causal attention (B=1, S=4096, D=1024, H=16, HKV=4, Dh=64, RoPE) on
8 Trainium2 NeuronCores.

Sharding: 8-way head parallelism. Core c owns query heads {2c, 2c+1} (which
share one KV head, g = c//2) and all 4096 query positions, so every core runs
the SAME program (one NEFF shared by all 8 cores); only the weight shards /
tables passed as inputs differ. Each core produces a partial output
projection [4096, 1024] (fp16, its heads' slice of wo); the host sums the 8
partials in float64.

Device program (causal fast path). Matmul operands are fp16 except the
attention-weight path (exp outputs and V), which is bf16 because the
unnormalized exp(score - 10) values span e^-54..e^+34 — beyond fp16 range.
All accumulation is f32 in PSUM. The ScalarE exp stream (~causal-area /
128 lanes) is the bottleneck engine; everything else is scheduled to hide
beneath it:

  projections: Q^T and a packed [K;V]^T GEMM from x^T, PSUM copied to SBUF
      fp16 on ScalarE, RoPE on DVE (rope(X) = X*cos + pairswap(X*sin_signed);
      pairswap = stream_shuffle over host-interleaved adjacent dh pairs).
      K rows are duplicated to partitions 64-127 with an SBUF-to-SBUF DMA so
      head 1's score matmuls see a matching base partition. V chunks are
      PE-transposed to V[kpos, dh] next to a memset ones column. Projection
      groups are interleaved with early attention tiles so exp starts ~12us
      into the program.

  attention (per q-tile of 1024, heads sequential, scores double-buffered,
      score matmuls emitted two chunks ahead): per key-chunk, S^T[k, q] =
      K^T_chunk^T @ Q^T (causal suffix only), exp on ScalarE reading PSUM
      (no row-max: scores are bounded well under exp overflow; fixed bias
      -10), within-chunk triangle zeroed by gpsimd affine_select. PV runs
      TRANSPOSED — out[q, dh+1] += P^T_chunk^T @ [V | 1] — so each 128-q
      chunk accumulates a [128, 65] PSUM region and the softmax denominator
      lands in column 64. PSUM start=True zeroes a whole 2KB bank, so each
      bank carries ONE accumulation group (start on first write, stop on
      last) and its four q-chunks are normalized together after the group
      closes: DVE reciprocal + per-partition tensor_scalar multiplies, PE
      transposes into the o-proj PSUM ring, DVE cast into ON.

  output projection: ON^T @ wo -> f32 PSUM, cast to fp16 (on DVE, or ScalarE
      once its exp stream has drained), DMA'd out. Units become eligible as
      their ON columns complete and are paced into the chunk stream.

If the mask input is NOT the standard causal mask, a dense fallback program
(the previous generation of this kernel: fp32r, untransposed PV, explicit
mask add) is compiled instead: slower, still correct for any additive mask.
"""

import os
from contextlib import ExitStack

import numpy as np

B, S, D = 1, 4096, 1024
H, HKV, DH = 16, 4, 64
HPC = 2             # query heads per core
NCORES = 8
ROPE_THETA = 10000.0
QT = 1024           # q columns per attention tile (causal fast path)
EXP_BIAS = -10.0    # shift inside exp; softmax-invariant, adds headroom

_cache = {}


def _build_causal():
    import concourse.bass as bass
    import concourse.tile as tile
    from concourse import bacc, mybir
    from concourse.masks import make_identity

    f32 = mybir.dt.float32
    bf16 = mybir.dt.bfloat16

    nc = bacc.Bacc(None, target_bir_lowering=False)

    NSG = S // 512                # 8 column groups in phase A
    NCH_D = D // 128              # 8 contraction chunks for projections
    NKCH = S // 128               # 32 key chunks
    NQT = S // QT                 # 4 q-tiles in phase B

    xTb = nc.dram_tensor("xTb", [D, S], bf16, kind="ExternalInput")
    # wqH/wkvH are host-packed [128, NCH_D*128]: partition = d-within-chunk,
    # so each loads with a single contiguous DMA
    wqH = nc.dram_tensor("wqH", [128, D], bf16, kind="ExternalInput")
    wkvH = nc.dram_tensor("wkvH", [128, D], bf16, kind="ExternalInput")
    woT = nc.dram_tensor("woT", [128, D], bf16, kind="ExternalInput")
    cosT = nc.dram_tensor("cosT", [128, S], bf16, kind="ExternalInput")
    sinTs = nc.dram_tensor("sinTs", [128, S], bf16, kind="ExternalInput")
    out = nc.dram_tensor("out", [S, D], bf16, kind="ExternalOutput")
    dbg = bool(os.environ.get("KDBG"))
    if dbg:
        dbg_qtr = nc.dram_tensor("dbg_qtr", [128, S], bf16, kind="ExternalOutput")
        dbg_ktr = nc.dram_tensor("dbg_ktr", [128, S], bf16, kind="ExternalOutput")
        dbg_on = nc.dram_tensor("dbg_on", [128, S], bf16, kind="ExternalOutput")

    with tile.TileContext(nc) as tc, ExitStack() as phase_a:
        with tc.tile_pool(name="const", bufs=1) as cpool, \
             tc.tile_pool(name="xs", bufs=2) as xs_pool, \
             tc.tile_pool(name="ab", bufs=2) as ab_pool, \
             tc.tile_pool(name="rtmp", bufs=2) as rtmp, \
             tc.tile_pool(name="esb", bufs=6) as e_pool, \
             tc.tile_pool(name="onp", bufs=6) as on_pool, \
             tc.tile_pool(name="rcp", bufs=2) as rc_pool, \
             tc.tile_pool(name="osb", bufs=6) as ob_pool, \
             tc.tile_pool(name="sps", bufs=1, space="PSUM") as sps, \
             tc.tile_pool(name="acc", bufs=1, space="PSUM") as acc_ps:
            prj_ps = phase_a.enter_context(
                tc.tile_pool(name="prj", bufs=1, space="PSUM"))

            # ---- resident constants ----
            wq_sb = cpool.tile([128, NCH_D, 128], bf16)
            wkv_sb = cpool.tile([128, NCH_D, 128], bf16)
            wo_sb = cpool.tile([128, D], bf16)
            cos_sb = cpool.tile([128, S], bf16)
            sin_sb = cpool.tile([128, S], bf16)
            QTr = cpool.tile([128, S], bf16)        # rope(Q)^T, 2 heads
            KTr = cpool.tile([128, S], bf16)        # rope(K)^T, duplicated
                                                    # rows (64-127 = 0-63)
            Vp = cpool.tile([128, NKCH, DH + 1], bf16)  # V chunks + ones col
            ON = cpool.tile([128, S], bf16)         # normalized O^T, 2 heads
            ident = cpool.tile([128, 128], bf16)
            biasc = cpool.tile([128, 1], f32)

            # startup-latency-critical DMA order: packed weights + the rope
            # table columns needed by sg0-1 first; x streams right behind
            nc.scalar.dma_start(out=wq_sb[:, :, :], in_=wqH[:, :])
            nc.scalar.dma_start(out=wkv_sb[:, :, :], in_=wkvH[:, :])
            nc.scalar.dma_start(out=cos_sb[:, 0:1024], in_=cosT[:, 0:1024])
            nc.scalar.dma_start(out=sin_sb[:, 0:1024], in_=sinTs[:, 0:1024])
            make_identity(nc, ident[:, :])
            # dummy transpose: starts the PE p-state ramp clock ~4us before
            # the first projection matmul, which then runs at full rate
            warm = sps.tile([128, 128], bf16, tag="s0", name="warm")
            nc.tensor.transpose(warm[:, :], ident[:, :], ident[:, :])
            nc.vector.memset(biasc, float(EXP_BIAS))
            nc.vector.memset(Vp[:, :, DH:DH + 1], 1.0)

            swap = [i ^ 1 for i in range(32)]

            def rope(src, out_ap, scols, np_):
                """out = src*cos + pairswap(src*sin_signed); all bf16 SBUF."""
                m1 = rtmp.tile([128, 512], bf16, tag="m1")
                m2 = rtmp.tile([128, 512], bf16, tag="m2")
                sh = rtmp.tile([128, 512], bf16, tag="sh")
                nc.vector.tensor_mul(m1[0:np_, :], src, cos_sb[0:np_, scols])
                nc.vector.tensor_mul(m2[0:np_, :], src, sin_sb[0:np_, scols])
                nc.vector.stream_shuffle(sh[0:np_, :], m2[0:np_, :], swap)
                nc.vector.tensor_add(out_ap, m1[0:np_, :], sh[0:np_, :])

            # ---- phase A: projections + rope + V transpose ----
            xsl = None
            for sg in range(NSG):
                scols = bass.ds(sg * 512, 512)
                if sg % 4 == 0:
                    xsl = xs_pool.tile([128, NCH_D, 2048], bf16, tag="x",
                                       name=f"x_{sg}")
                    for cd in range(NCH_D):
                        nc.sync.dma_start(
                            out=xsl[:, cd, :],
                            in_=xTb[cd * 128:(cd + 1) * 128,
                                    sg * 512:sg * 512 + 2048])
                xoff = (sg % 4) * 512
                qt_ps = prj_ps.tile([128, 512], f32, tag="qt")
                kv_ps = prj_ps.tile([128, 512], f32, tag="kv")
                for cd in range(NCH_D):
                    st, sp = (cd == 0), (cd == NCH_D - 1)
                    rhs = xsl[:, cd, xoff:xoff + 512]
                    nc.tensor.matmul(qt_ps[:, :], wq_sb[:, cd, :], rhs,
                                     start=st, stop=sp)
                    nc.tensor.matmul(kv_ps[:, :], wkv_sb[:, cd, :], rhs,
                                     start=st, stop=sp)
                qt_sb = ab_pool.tile([128, 512], bf16, tag="qt")
                kv_sb = ab_pool.tile([128, 512], bf16, tag="kv")
                nc.scalar.copy(qt_sb, qt_ps[:, :])
                nc.scalar.copy(kv_sb, kv_ps[:, :])
                rope(qt_sb[:, :], QTr[:, scols], scols, 128)
                rope(kv_sb[0:64, :], KTr[0:64, scols], scols, 64)
                # head 1 reads its K copy at partitions 64-127 (score matmul
                # operands must share a base partition with Q head 1)
                nc.scalar.dma_start(out=KTr[64:128, scols],
                                    in_=KTr[0:64, scols])
                for j in range(4):
                    vt = prj_ps.tile([128, DH], bf16, tag="vt")
                    nc.tensor.transpose(vt[:, :],
                                        kv_sb[64:128, j * 128:(j + 1) * 128],
                                        ident[64:128, 64:128])
                    nc.vector.tensor_copy(Vp[:, sg * 4 + j, 0:DH], vt[:, :])

            if dbg:
                nc.sync.dma_start(out=dbg_qtr[:, :], in_=QTr[:, :])
                nc.sync.dma_start(out=dbg_ktr[:, :], in_=KTr[:, :])

            # ---- phase B: attention + interleaved output projection ----
            phase_a.close()
            phase_b = ExitStack()
            sps = phase_b.enter_context(
                tc.tile_pool(name="sps", bufs=1, space="PSUM"))
            acc_ps = phase_b.enter_context(
                tc.tile_pool(name="acc", bufs=1, space="PSUM"))
            op_ps = phase_b.enter_context(
                tc.tile_pool(name="ops", bufs=2, space="PSUM"))

            pending = []  # deferred o-proj units of the previous tile

            def emit_oproj(g128, dseg):
                op = op_ps.tile([128, 512], f32, tag="op",
                                name=f"op_{g128}_{dseg}")
                nc.tensor.matmul(op[:, :],
                                 ON[:, g128 * 128:(g128 + 1) * 128],
                                 wo_sb[:, dseg * 512:(dseg + 1) * 512],
                                 start=True, stop=True)
                ob = ob_pool.tile([128, 512], bf16, tag="ob",
                                  name=f"ob_{g128}_{dseg}")
                nc.vector.tensor_copy(ob, op[:, :])
                nc.sync.dma_start(
                    out=out[g128 * 128:(g128 + 1) * 128,
                            dseg * 512:(dseg + 1) * 512],
                    in_=ob)

            # global chunk stream for one-ahead score emission
            stream = [(t, h, c)
                      for t in range(NQT)
                      for h in range(HPC)
                      for c in range(8 * (t + 1))]

            s_tiles = {}
            sidx = 0  # alternator for score psum tags

            def emit_score(key):
                nonlocal sidx
                t, h, c = key
                q0 = t * QT
                qs = max(0, c * 128 - q0)
                s_ps = sps.tile([128, QT], f32, tag=f"s{sidx % 2}",
                                name=f"s_{t}_{h}_{c}")
                sidx += 1
                lhs = KTr[64 * h:64 * h + 64, c * 128:(c + 1) * 128]
                for lo, hi in ((qs, 512), (max(qs, 512), QT)):
                    if lo >= hi:
                        continue
                    nc.tensor.matmul(s_ps[:, lo:hi], lhs,
                                     QTr[64 * h:64 * h + 64, q0 + lo:q0 + hi],
                                     start=True, stop=True)
                s_tiles[key] = s_ps

            emit_score(stream[0])
            cur = None  # per-(t,h) state: (oacc, on_t, rc)
            for i, key in enumerate(stream):
                t, h, c = key
                q0 = t * QT
                qs = max(0, c * 128 - q0)
                if i + 1 < len(stream):
                    emit_score(stream[i + 1])
                if c == 0:
                    oacc = acc_ps.tile([128, 2, 512], f32, tag="acc",
                                       name=f"oacc_{t}_{h}")
                    on_t = on_pool.tile([128, 8, DH], bf16, tag="on",
                                        name=f"on_{t}_{h}")
                    rc = rc_pool.tile([128, 8], f32, tag="rc",
                                      name=f"rc_{t}_{h}")
                    cur = (oacc, on_t, rc)
                else:
                    oacc, on_t, rc = cur
                s_ps = s_tiles.pop(key)
                e_sb = e_pool.tile([128, QT], bf16, tag=f"e{h}",
                                   name=f"e_{t}_{h}_{c}")
                nc.scalar.activation(e_sb[:, qs:QT], s_ps[:, qs:QT],
                                     mybir.ActivationFunctionType.Exp,
                                     bias=biasc[:, :], scale=1.0)
                if c * 128 >= q0:
                    nc.gpsimd.affine_select(
                        out=e_sb[:, qs:qs + 128], in_=e_sb[:, qs:qs + 128],
                        pattern=[[1, 128]],
                        compare_op=mybir.AluOpType.is_ge,
                        fill=0.0, base=0, channel_multiplier=-1)
                for qc in range(qs // 128, 8):
                    lastc = t * 8 + qc
                    if c > lastc:
                        continue
                    b, qq = divmod(qc, 4)
                    reg = oacc[:, b, qq * 65:qq * 65 + 65]
                    nc.tensor.matmul(reg,
                                     e_sb[:, qc * 128:(qc + 1) * 128],
                                     Vp[:, c, :],
                                     start=(c == 0), stop=(c == lastc))
                    if c == lastc:
                        nc.vector.reciprocal_approx_fast(
                            rc[:, qc:qc + 1],
                            oacc[:, b, qq * 65 + DH:qq * 65 + DH + 1])
                        nc.vector.tensor_scalar_mul(
                            on_t[:, qc, :], oacc[:, b, qq * 65:qq * 65 + DH],
                            rc[:, qc:qc + 1])
                        if qc in (3, 7):
                            trp = op_ps.tile([DH, 512], bf16, tag="op",
                                             name=f"trp_{t}_{h}_{qc}")
                            for k in range(4):
                                qc2 = qc - 3 + k
                                nc.tensor.transpose(
                                    trp[:, k * 128:(k + 1) * 128],
                                    on_t[:, qc2, :], ident[:, :])
                            nc.vector.tensor_copy(
                                ON[64 * h:64 * h + 64,
                                   q0 + (qc - 3) * 128:q0 + (qc + 1) * 128],
                                trp[:, :])
                if pending and c >= 2:
                    emit_oproj(*pending.pop(0))
                if h == 1 and c == 8 * (t + 1) - 1:
                    for qsub in range(QT // 128):
                        for dseg in range(D // 512):
                            pending.append((t * (QT // 128) + qsub, dseg))
            if dbg:
                nc.sync.dma_start(out=dbg_on[:, :], in_=ON[:, :])
            while pending:
                emit_oproj(*pending.pop(0))
            phase_b.close()

    nc.compile()
    return nc


def _rope_tables(dtype):
    """[128, S] cos / sign-adjusted sin tables in pair-interleaved dh order,
    tiled for the two query heads."""
    inv_freq = 1.0 / (ROPE_THETA ** (np.arange(0, DH, 2, dtype=np.float64) / DH))
    ang = np.arange(S, dtype=np.float64)[:, None] * inv_freq[None, :]  # [S, 32]
    cosv = np.cos(ang)
    sinv = np.sin(ang)
    C64 = np.empty((DH, S), dtype=np.float64)
    S64 = np.empty((DH, S), dtype=np.float64)
    for j in range(DH):
        C64[j] = cosv[:, j // 2]
        S64[j] = sinv[:, j // 2] * (1.0 if j % 2 == 0 else -1.0)
    cosT = np.ascontiguousarray(np.tile(C64, (2, 1)).astype(dtype))
    sinTs = np.ascontiguousarray(np.tile(S64, (2, 1)).astype(dtype))
    return cosT, sinTs


def _host_inputs_causal(x, wq, wk, wv, wo):
    """Build the 8 per-core input dicts (bf16)."""
    from ml_dtypes import bfloat16

    x2 = np.asarray(x, dtype=np.float32).reshape(S, D)
    xTb = np.ascontiguousarray(x2.T.astype(bfloat16))

    # rope pair-interleaved dh order: [0, 32, 1, 33, ...]
    perm = np.empty(DH, dtype=np.int64)
    perm[0::2] = np.arange(DH // 2)
    perm[1::2] = np.arange(DH // 2) + DH // 2

    cosT, sinTs = _rope_tables(bfloat16)

    wq4 = np.asarray(wq, dtype=np.float32).reshape(H, DH, D)
    wk4 = np.asarray(wk, dtype=np.float32).reshape(HKV, DH, D)
    wv4 = np.asarray(wv, dtype=np.float32).reshape(HKV, DH, D)
    wo2 = np.asarray(wo, dtype=np.float32)

    ins = []
    for c in range(NCORES):
        h0, h1 = 2 * c, 2 * c + 1
        g = h0 // (H // HKV)
        wq_c = np.concatenate([wq4[h0][perm], wq4[h1][perm]], axis=0)  # [128, D]
        wkv_c = np.concatenate([wk4[g][perm], wv4[g]], axis=0)         # [128, D]
        wo_c = wo2[:, np.r_[h0 * DH:(h0 + 1) * DH, h1 * DH:(h1 + 1) * DH]]

        def pack(w):  # [128 out-dims, D] -> [128 d-in-chunk, NCH * 128 out]
            return np.ascontiguousarray(
                w.T.reshape(8, 128, 128).transpose(1, 0, 2).reshape(128, D)
                .astype(bfloat16))

        ins.append({
            "xTb": xTb,
            "wqH": pack(wq_c),
            "wkvH": pack(wkv_c),
            "woT": np.ascontiguousarray(wo_c.T.astype(bfloat16)),
            "cosT": cosT,
            "sinTs": sinTs,
        })
    return ins


# ---------------------------------------------------------------------------
# Dense fallback (any additive mask): previous-generation program, fp32r.
# ---------------------------------------------------------------------------

def _build_dense():
    import concourse.bass as bass
    import concourse.tile as tile
    from concourse import bacc, mybir
    from concourse.masks import make_identity

    f32 = mybir.dt.float32
    f16 = mybir.dt.float16
    f32r = mybir.dt.float32r

    nc = bacc.Bacc(None, target_bir_lowering=False)

    QT_TILE = 1024
    NSG = S // 512
    NCH_D = D // 128
    NKCH = S // 128
    NQT = S // QT_TILE

    xT = nc.dram_tensor("xT", [D, S], f32r, kind="ExternalInput")
    wqT = nc.dram_tensor("wqT", [D, 128], f32r, kind="ExternalInput")
    wkTd = nc.dram_tensor("wkTd", [D, 128], f32r, kind="ExternalInput")
    wvT = nc.dram_tensor("wvT", [D, DH], f32r, kind="ExternalInput")
    woT = nc.dram_tensor("woT", [128, D], f32r, kind="ExternalInput")
    cosT = nc.dram_tensor("cosT", [128, S], f32, kind="ExternalInput")
    sinTs = nc.dram_tensor("sinTs", [128, S], f32, kind="ExternalInput")
    maskT = nc.dram_tensor("maskT", [S, S], f32, kind="ExternalInput")
    out = nc.dram_tensor("out", [S, D], f16, kind="ExternalOutput")

    with tile.TileContext(nc) as tc, ExitStack() as phase_a:
        with tc.tile_pool(name="const", bufs=1) as cpool, \
             tc.tile_pool(name="xs", bufs=4) as xs_pool, \
             tc.tile_pool(name="rtmp", bufs=2) as rtmp, \
             tc.tile_pool(name="vtt", bufs=2) as vtt_pool, \
             tc.tile_pool(name="esb", bufs=2) as e_pool, \
             tc.tile_pool(name="osb", bufs=2) as ot_pool, \
             tc.tile_pool(name="mtile", bufs=2) as m_pool:
            prj_ps = phase_a.enter_context(tc.tile_pool(name="prj", bufs=2, space="PSUM"))
            trp_ps = phase_a.enter_context(tc.tile_pool(name="trp", bufs=2, space="PSUM"))

            wq_sb = cpool.tile([128, NCH_D, 128], f32r)
            wk_sb = cpool.tile([128, NCH_D, 128], f32r)
            wv_sb = cpool.tile([128, NCH_D, DH], f32r)
            wo_sb = cpool.tile([128, D], f32r)
            cos_sb = cpool.tile([128, S], f32)
            sin_sb = cpool.tile([128, S], f32)
            QTr = cpool.tile([128, S], f32r)
            KTr = cpool.tile([128, S], f32r)
            Vp = cpool.tile([128, NKCH, DH + 1], f32r)
            ON = cpool.tile([128, S], f32r)
            ident = cpool.tile([DH, DH], f32)
            ones_row = cpool.tile([128, DH], f32)
            biasc = cpool.tile([128, 1], f32)

            for cd in range(NCH_D):
                nc.scalar.dma_start(out=wq_sb[:, cd, :], in_=wqT[cd * 128:(cd + 1) * 128, :])
                nc.scalar.dma_start(out=wk_sb[:, cd, :], in_=wkTd[cd * 128:(cd + 1) * 128, :])
                nc.scalar.dma_start(out=wv_sb[:, cd, :], in_=wvT[cd * 128:(cd + 1) * 128, :])
            for sg in range(NSG):
                sl = bass.ds(sg * 512, 512)
                nc.scalar.dma_start(out=cos_sb[:, sl], in_=cosT[:, sg * 512:(sg + 1) * 512])
                nc.scalar.dma_start(out=sin_sb[:, sl], in_=sinTs[:, sg * 512:(sg + 1) * 512])
            nc.scalar.dma_start(out=wo_sb, in_=woT[:, :])
            make_identity(nc, ident[:, :])
            nc.vector.memset(ones_row, 1.0)
            nc.vector.memset(biasc, float(EXP_BIAS))
            nc.vector.memset(Vp[:, :, DH:DH + 1].bitcast(f32), 1.0)

            def rope_from_psum(ps_ap, sb_out_ap, scols, width):
                m1 = rtmp.tile([128, 512], f32, tag="rope_m1")
                m2 = rtmp.tile([128, 512], f32, tag="rope_m2")
                sh = rtmp.tile([128, 512], f32, tag="rope_sh")
                nc.vector.tensor_mul(m1[:, :width], ps_ap, cos_sb[:, scols])
                nc.vector.tensor_mul(m2[:, :width], ps_ap, sin_sb[:, scols])
                nc.vector.stream_shuffle(sh[:, :width], m2[:, :width],
                                         [i ^ 1 for i in range(32)])
                nc.vector.tensor_add(sb_out_ap, m1[:, :width], sh[:, :width])

            for sg in range(NSG):
                scols = bass.ds(sg * 512, 512)
                qt_ps = prj_ps.tile([128, 512], f32, tag="qt")
                kt_ps = prj_ps.tile([128, 512], f32, tag="kt")
                vt_ps = prj_ps.tile([DH, 512], f32, tag="vt")
                for cd in range(NCH_D):
                    xsl = xs_pool.tile([128, 512], f32r)
                    nc.sync.dma_start(out=xsl, in_=xT[cd * 128:(cd + 1) * 128,
                                                      sg * 512:(sg + 1) * 512])
                    st = (cd == 0)
                    sp = (cd == NCH_D - 1)
                    nc.tensor.matmul(qt_ps[:, :], wq_sb[:, cd, :], xsl,
                                     start=st, stop=sp)
                    nc.tensor.matmul(kt_ps[:, :], wk_sb[:, cd, :], xsl,
                                     start=st, stop=sp)
                    nc.tensor.matmul(vt_ps[:, :], wv_sb[:, cd, :], xsl,
                                     start=st, stop=sp)
                rope_from_psum(qt_ps[:, :], QTr[:, scols], scols, 512)
                rope_from_psum(kt_ps[:, :], KTr[:, scols], scols, 512)
                vt_sb = vtt_pool.tile([DH, 512], f32)
                nc.scalar.copy(vt_sb, vt_ps[:, :])
                for j in range(4):
                    kc = sg * 4 + j
                    tr = trp_ps.tile([128, DH], f32)
                    nc.tensor.transpose(tr[:, :], vt_sb[:, j * 128:(j + 1) * 128],
                                        ident[:, :])
                    nc.vector.tensor_copy(Vp[:, kc, 0:DH], tr[:, :])

            phase_a.close()
            phase_b = ExitStack()
            s_ps_pool = phase_b.enter_context(tc.tile_pool(name="sps", bufs=1, space="PSUM"))
            o_ps_pool = phase_b.enter_context(tc.tile_pool(name="ops", bufs=1, space="PSUM"))

            def emit_oproj(qsub, dseg):
                op = o_ps_pool.tile([128, 512], f32, tag=f"op{dseg}",
                                    name=f"op_{qsub}_{dseg}")
                nc.tensor.matmul(
                    op[:, :],
                    ON[:, qsub * 128:(qsub + 1) * 128],
                    wo_sb[:, dseg * 512:(dseg + 1) * 512],
                    start=True, stop=True)
                ob = m_pool.tile([128, 512], f16, tag="ostage")
                nc.vector.tensor_copy(ob, op[:, :])
                nc.sync.dma_start(
                    out=out[qsub * 128:(qsub + 1) * 128,
                            dseg * 512:(dseg + 1) * 512],
                    in_=ob)

            pending = []
            for t in range(NQT):
                q0 = t * QT_TILE
                for h in range(HPC):
                    o_ps = o_ps_pool.tile([DH + 1, QT_TILE], f32, tag="oacc",
                                          name=f"ops_{t}_{h}")
                    for ci in range(NKCH):
                        c = ci
                        s_ps = s_ps_pool.tile([128, QT_TILE], f32,
                                              tag=f"s{ci % 2}",
                                              name=f"s_{t}_{h}_{ci}")
                        lhs = KTr[64 * h:64 * h + 64, c * 128:(c + 1) * 128]
                        for lo, hi in ((0, 512), (512, QT_TILE)):
                            nc.tensor.matmul(
                                s_ps[:, lo:hi], lhs,
                                QTr[64 * h:64 * h + 64, q0 + lo:q0 + hi],
                                start=True, stop=True)
                        sm = m_pool.tile([128, QT_TILE], f32, tag="mask")
                        nc.sync.dma_start(
                            out=sm, in_=maskT[c * 128:(c + 1) * 128,
                                              q0:q0 + QT_TILE])
                        sms = m_pool.tile([128, QT_TILE], f32, tag="masked")
                        nc.vector.tensor_add(sms, s_ps[:, :], sm)
                        e_sb = e_pool.tile([128, QT_TILE], f32r, tag=f"e{h}",
                                           name=f"e_{t}_{h}_{ci}")
                        nc.scalar.activation(
                            e_sb[:, :], sms,
                            mybir.ActivationFunctionType.Exp,
                            bias=biasc[:, :], scale=1.0)
                        for lo, hi in ((0, 512), (512, QT_TILE)):
                            nc.tensor.matmul(
                                o_ps[:, lo:hi], Vp[:, c, :],
                                e_sb[:, lo:hi],
                                start=(c == 0), stop=(c == NKCH - 1))
                        if pending and ci >= 2:
                            emit_oproj(*pending.pop(0))
                    ot = ot_pool.tile([DH + 1, QT_TILE], f32, tag="ot",
                                      name=f"ot_{t}_{h}")
                    nc.vector.tensor_copy(ot, o_ps[:, :])
                    rcp = ot_pool.tile([DH + 1, QT_TILE], f32, tag="rc",
                                       name=f"rc_{t}_{h}")
                    nc.vector.reciprocal_approx_fast(rcp, ot[:, :])
                    for seg in range(QT_TILE // 512):
                        cs = bass.ds(seg * 512, 512)
                        rbseg = o_ps_pool.tile([128, 512], f32, tag=f"op{seg}",
                                               name=f"rb_{t}_{h}_{seg}")
                        nc.tensor.matmul(rbseg[0:DH, :],
                                         ones_row[DH:DH + 1, :],
                                         rcp[DH:DH + 1, cs],
                                         start=True, stop=True)
                        nc.vector.tensor_mul(
                            ON[64 * h:64 * h + 64,
                               q0 + seg * 512:q0 + (seg + 1) * 512],
                            ot[0:DH, seg * 512:(seg + 1) * 512], rbseg[0:DH, :])
                for j in range(QT_TILE // 128):
                    for dseg in range(D // 512):
                        pending.append((t * (QT_TILE // 128) + j, dseg))
            while pending:
                emit_oproj(*pending.pop(0))
            phase_b.close()

    nc.compile()
    return nc


def _host_inputs_dense(x, mask, wq, wk, wv, wo):
    x2 = np.asarray(x, dtype=np.float32).reshape(S, D)
    xT = np.ascontiguousarray(x2.T)

    perm = np.empty(DH, dtype=np.int64)
    perm[0::2] = np.arange(DH // 2)
    perm[1::2] = np.arange(DH // 2) + DH // 2

    cosT, sinTs = _rope_tables(np.float32)

    wq4 = np.asarray(wq, dtype=np.float32).reshape(H, DH, D)
    wk4 = np.asarray(wk, dtype=np.float32).reshape(HKV, DH, D)
    wv4 = np.asarray(wv, dtype=np.float32).reshape(HKV, DH, D)
    wo2 = np.asarray(wo, dtype=np.float32)
    maskT = np.ascontiguousarray(np.asarray(mask, dtype=np.float32).T)

    ins = []
    for c in range(NCORES):
        h0, h1 = 2 * c, 2 * c + 1
        g = h0 // (H // HKV)
        wq_c = np.concatenate([wq4[h0][perm], wq4[h1][perm]], axis=0)
        wk_c = np.concatenate([wk4[g][perm], wk4[g][perm]], axis=0)
        wo_c = wo2[:, np.r_[h0 * DH:(h0 + 1) * DH, h1 * DH:(h1 + 1) * DH]]
        ins.append({
            "xT": xT,
            "wqT": np.ascontiguousarray(wq_c.T),
            "wkTd": np.ascontiguousarray(wk_c.T),
            "wvT": np.ascontiguousarray(wv4[g].T),
            "woT": np.ascontiguousarray(wo_c.T),
            "cosT": cosT,
            "sinTs": sinTs,
            "maskT": maskT,
        })
    return ins


def _build(causal):
    return _build_causal() if causal else _build_dense()


def _is_causal(mask):
    mask = np.asarray(mask)
    if mask.shape != (S, S):
        return False
    expected = np.where(np.tril(np.ones((S, S), dtype=bool)), np.float32(0.0),
                        np.float32(-1e9))
    return np.array_equal(mask, expected)


def run_cores(x, mask, wq, wk, wv, wo, **spmd_kwargs):
    """Compile (cached) + run on 8 cores; returns BassKernelResults."""
    from concourse.bass_utils import run_bass_kernel_spmd

    causal = _is_causal(mask)
    if causal not in _cache:
        _cache[causal] = _build_causal() if causal else _build_dense()
    nc = _cache[causal]

    if causal:
        ins = _host_inputs_causal(x, wq, wk, wv, wo)
    else:
        ins = _host_inputs_dense(x, mask, wq, wk, wv, wo)
    return run_bass_kernel_spmd(nc, ins, core_ids=list(range(NCORES)),
                                **spmd_kwargs)


def kernel(x, mask, wq, wk, wv, wo):
    res = run_cores(x, mask, wq, wk, wv, wo)
    acc = np.zeros((S, D), dtype=np.float64)
    for r in res.results:
        acc += r["out"].astype(np.float64)
    return acc.astype(np.float32).reshape(B, S, D)
